# revision 45
# baseline (speedup 1.0000x reference)
"""Distributed causal attention for TRN2 (8 NeuronCores).

Reference op (per core-external semantics):
    qkv = x @ w_qkv + b_qkv ; split into per-head q,k,v (16 heads, hd=64)
    causal softmax(q k^T / 8) v per head ; concat heads ; out = . @ w_proj + b_proj

Sharding: head-parallel attention (2 heads/core), AllToAll redistribution to
sequence-parallel for the output projection (each core owns S/8 query rows).

v2 structure (vs v1):
  - qkv projection is emitted per-seq-block interleaved with head-0 attention
    so the Scalar engine (exp, the critical resource) starts early.
  - QK^T matmuls alternate PE row-groups (even tiles use the head's natural
    64 partitions, odd tiles a DMA-duplicated copy in the opposite half) so
    consecutive K=64 matmuls run concurrently in the PE array.
  - softmax normalize+stage happens per q-block (reciprocal_approx_fast),
    not per window, so the A2As fire immediately after their last block.
  - head 1 runs in two 256-column windows with separate A2As; the output
    projection is split at K=64 so head-0 terms accumulate during the last
    A2A and only head-1-w1 terms remain after it.
  - qkv bias is applied by the DVE during PSUM->SBUF eviction (per-partition
    scalar), not by K=1 matmuls.

All matmuls run in bf16 (fp32 PSUM accumulation); softmax runs without
max-subtraction (scores are bounded for this problem's scale), with
denominators via a ones-column appended to V.

kernel(**inputs) takes the FULL fp32 inputs and returns the FULL fp32 output.
"""

import os

import numpy as np
import ml_dtypes

# comma-separated debug kill-switches, e.g. KBISECT=recip,bias,dup
KBISECT = set(filter(None, os.environ.get("KBISECT", "").split(",")))

import concourse.bacc as bacc
import concourse.bass as bass
import concourse.tile as tile
from concourse import masks, mybir
from concourse.bass_utils import run_bass_kernel_spmd

N_CORES = 8
D = 1024
H = 16
HD = 64
HPC = H // N_CORES          # heads per core = 2
MQKV = 3 * HPC * HD         # per-core qkv feature cols = 384

BF16 = mybir.dt.bfloat16
F32 = mybir.dt.float32
bf16 = ml_dtypes.bfloat16

# Bumping this changes the compiled executable's signature (a dummy input's
# shape encodes it), forcing a fresh compile + stage. Bump if a crashed run
# leaves a poisoned staged executable behind.
BUILD_SALT = 17 + sum(len(f) for f in KBISECT)


def build(S):
    QB = S // N_CORES        # query rows per core (A2A shard) = 512
    NQ = N_CORES             # number of q blocks == cores
    SKT = S // 128           # total sk tiles
    NPROJ = S // 512         # qkv-proj seq blocks of 512
    MT = QB // 128           # out-row subtiles of 128

    nc = bacc.Bacc("TRN2", num_devices=N_CORES)

    xT = nc.declare_dram_parameter("xT", [D, S], BF16, isOutput=False)
    wqkv = nc.declare_dram_parameter("wqkv", [D, MQKV], BF16, isOutput=False)
    bqkv = nc.declare_dram_parameter("bqkv", [1, MQKV], BF16, isOutput=False)
    bqcol = nc.declare_dram_parameter("bqcol", [128, 3], F32, isOutput=False)
    wproj = nc.declare_dram_parameter("wproj", [D, D], BF16, isOutput=False)
    bproj = nc.declare_dram_parameter("bproj", [1, D], BF16, isOutput=False)
    maskp = nc.declare_dram_parameter("mask", [128, 1024], BF16, isOutput=False)
    salt = nc.declare_dram_parameter("salt", [1, BUILD_SALT], F32, isOutput=False)
    out_ext = nc.declare_dram_parameter("out", [QB, D], F32, isOutput=True)

    # collective staging: head0 full-width; head1 in two 256-col windows
    a2a_in0 = nc.dram_tensor("a2a_in0", [NQ, HD, QB], BF16)
    a2a_out0 = nc.dram_tensor("a2a_out0", [NQ, HD, QB], BF16)
    WIN1 = [(0, 256), (256, 128), (384, 128)]   # head-1 windows (q0, qw)
    a2a_in1 = [nc.dram_tensor(f"a2a_in1_{w}", [NQ, HD, qw], BF16)
               for w, (q0, qw) in enumerate(WIN1)]
    a2a_out1 = [nc.dram_tensor(f"a2a_out1_{w}", [NQ, HD, qw], BF16)
                for w, (q0, qw) in enumerate(WIN1)]
    rden_dram = nc.dram_tensor("rden_dram", [HPC, NQ, QB], F32)
    den_dram = nc.dram_tensor("den_dram", [HPC, NQ, QB], F32)

    with tile.TileContext(nc) as tc:
        with (
            tc.tile_pool(name="singles", bufs=1) as singles,
            tc.tile_pool(name="xpool", bufs=3) as xpool,
            tc.tile_pool(name="work", bufs=2) as work,
            tc.tile_pool(name="norm", bufs=6) as norm,
            tc.tile_pool(name="ppool", bufs=10) as ppool,
            tc.tile_pool(name="upool", bufs=6) as upool,
            tc.tile_pool(name="ps1", bufs=3, space="PSUM") as ps1,
            tc.tile_pool(name="ps2", bufs=2, space="PSUM") as ps2,
        ):
            # ---- constants / weights ----
            w_sb = singles.tile([128, 8, MQKV], BF16)
            nc.sync.dma_start(out=w_sb[:], in_=wqkv.rearrange("(a p) m -> p a m", p=128))
            bq_sb = singles.tile([1, MQKV], BF16)
            nc.sync.dma_start(out=bq_sb[:], in_=bqkv[:])
            # bias as a per-partition f32 column for tensor_scalar eviction:
            # bq_col[p, m] = b_qkv[m*128 + p]
            bq_col = singles.tile([128, 3], F32)
            nc.sync.dma_start(out=bq_col[:], in_=bqcol[:])
            mask_sb = singles.tile([128, 1024], BF16)
            nc.sync.dma_start(out=mask_sb[:], in_=maskp[:])
            ones_sb = singles.tile([1, 512], BF16)
            nc.vector.memset(ones_sb[:], 1.0)
            ident = singles.tile([128, 128], BF16)
            masks.make_identity(nc, ident[:])
            wp_sb = singles.tile([128, 8, D], BF16)
            nc.sync.dma_start(out=wp_sb[:], in_=wproj.rearrange("(a p) m -> p a m", p=128))

            bp_sb = singles.tile([1, D], BF16)
            nc.sync.dma_start(out=bp_sb[:], in_=bproj[:])
            # b_proj broadcast to all 128 partitions (free-dim bias for the
            # output rows; folded into the PSUM eviction add)
            bp_full = singles.tile([128, D], BF16)
            nc.sync.dma_start(
                out=bp_full[:],
                in_=bass.AP(tensor=bproj, offset=0, ap=[[0, 128], [1, D]]),
            )
            salt_sb = singles.tile([1, BUILD_SALT], F32)
            nc.sync.dma_start(out=salt_sb[:], in_=salt[:])

            # persistent activation tensors
            qkvT = singles.tile([128, 3, S], BF16)   # [feat(2 heads), {q,k,v}, seq]
            # cross-duplicates: kdup rows 64:128 = k_h0, rows 0:64 = k_h1
            kdup = singles.tile([128, S], BF16)
            qdup = singles.tile([128, S], BF16)
            v_sb = singles.tile([128, SKT, 2 * (HD + 1)], BF16)
            nc.vector.memset(v_sb[:, :, HD:HD + 1], 1.0)
            nc.vector.memset(v_sb[:, :, 2 * HD + 1:2 * HD + 2], 1.0)
            # gathered A2A results: head 0 rows 0-63, head 1 rows 64-127
            ao = singles.tile([128, NQ, QB], BF16)

            xT_r = xT.rearrange("(a p) s -> p a s", p=128)
            x_tiles = {}

            def load_x(n):
                if n >= NPROJ:
                    return
                xs = xpool.tile([128, 8, 512], BF16, tag="x")
                for a in range(8):
                    nc.sync.dma_start(
                        out=xs[:, a, :], in_=xT_r[:, a, 512 * n:512 * (n + 1)]
                    )
                x_tiles[n] = xs

            def proj_block(n):
                """qkv^T projection for seq block n: qkvT[:, :, 512n:512n+512]."""
                xs = x_tiles.pop(n)
                for m in range(3):
                    ps = ps1.tile([128, 1024], F32, tag="ps1")
                    for a in range(8):
                        nc.tensor.matmul(
                            ps[:, 0:512],
                            lhsT=w_sb[:, a, 128 * m:128 * (m + 1)],
                            rhs=xs[:, a, :],
                            start=(a == 0), stop=(a == 7),
                        )
                    nc.vector.tensor_scalar_add(
                        qkvT[:, m, 512 * n:512 * (n + 1)], ps[:, 0:512],
                        bq_col[:, m:m + 1],
                    )
                # cross-duplicate k and q halves for PE row-group alternation
                if "dup" not in KBISECT:
                    nc.gpsimd.dma_start(
                        out=kdup[64:128, 512 * n:512 * (n + 1)],
                        in_=qkvT[0:64, 1, 512 * n:512 * (n + 1)],
                    )
                    nc.gpsimd.dma_start(
                        out=kdup[0:64, 512 * n:512 * (n + 1)],
                        in_=qkvT[64:128, 1, 512 * n:512 * (n + 1)],
                    )
                    nc.gpsimd.dma_start(
                        out=qdup[64:128, 512 * n:512 * (n + 1)],
                        in_=qkvT[0:64, 0, 512 * n:512 * (n + 1)],
                    )
                    nc.gpsimd.dma_start(
                        out=qdup[0:64, 512 * n:512 * (n + 1)],
                        in_=qkvT[64:128, 0, 512 * n:512 * (n + 1)],
                    )
                # V natural layout for the 4 new sk tiles
                for t in range(4 * n, 4 * n + 4):
                    pt = ps2.tile([128, 128], BF16, tag="ps2")
                    nc.tensor.transpose(pt[:], qkvT[:, 2, 128 * t:128 * (t + 1)], ident[:])
                    nc.vector.tensor_copy(v_sb[:, t, 0:HD], pt[:, 0:HD])
                    nc.vector.tensor_copy(v_sb[:, t, HD + 1:2 * HD + 1], pt[:, HD:2 * HD])

            def k_ap(h, t):
                """lhsT for QK^T: head h, sk tile t, alternating row halves."""
                lo, hi = (0, 64) if h == 0 else (64, 128)
                if t % 2 == 0 or "dup" in KBISECT:
                    return qkvT[lo:hi, 1, 128 * t:128 * (t + 1)]
                olo = 64 - lo
                return kdup[olo:olo + 64, 128 * t:128 * (t + 1)]

            def q_ap(h, t, c0, cw):
                """rhs for QK^T: head h q cols [c0, c0+cw), matching k_ap rows."""
                lo, hi = (0, 64) if h == 0 else (64, 128)
                if t % 2 == 0 or "dup" in KBISECT:
                    return qkvT[lo:hi, 0, c0:c0 + cw]
                olo = 64 - lo
                return qdup[olo:olo + 64, c0:c0 + cw]

            def attn_block(h, qb, q0, qw, gsz):
                """Attention for head h, q cols [qb*QB+q0, +qw); gsz sk-tiles
                per exp group (gsz*qw == 1024). Returns staged output."""
                nk = (qb * QB + q0 + qw) // 128  # causal sk tiles
                dstart = (qb * QB + q0) // 128   # first (partially) masked tile
                # tile j of a group goes to PSUM bank j%2 at column (j//2)*qw:
                # consecutive tiles alternate PE row-groups and run
                # concurrently, so they must drain into different banks.
                p_tiles = []
                for g0 in range(0, nk, gsz):
                    gw = min(gsz, nk - g0)
                    ps = ps1.tile([128, 2, 512], F32, tag="ps1")
                    for j in range(gw):
                        t = g0 + j
                        b, c = j % 2, (j // 2) * qw
                        nc.tensor.matmul(
                            ps[:, b, c:c + qw],
                            lhsT=k_ap(h, t),
                            rhs=q_ap(h, t, qb * QB + q0, qw),
                            start=True, stop=True,
                        )
                    pt = ppool.tile([128, 2, 512], BF16, tag="p")
                    pairs = gw // 2
                    if pairs:
                        nc.scalar.activation(
                            pt[:, :, 0:pairs * qw], ps[:, :, 0:pairs * qw],
                            mybir.ActivationFunctionType.Exp, scale=0.125,
                        )
                    if gw % 2:
                        nc.scalar.activation(
                            pt[:, 0, pairs * qw:(pairs + 1) * qw],
                            ps[:, 0, pairs * qw:(pairs + 1) * qw],
                            mybir.ActivationFunctionType.Exp, scale=0.125,
                        )
                    p_tiles.append(pt)
                # causal mask on diagonal tiles: tile dstart+d needs
                # mask[r, j] = (r <= j - 128*d) over j in [0, qw)
                for t in range(dstart, nk):
                    d = t - dstart
                    g0, j = divmod(t, gsz)
                    b, c = j % 2, (j // 2) * qw
                    c0 = 384 - 128 * d
                    nc.vector.tensor_mul(
                        p_tiles[g0][:, b, c:c + qw], p_tiles[g0][:, b, c:c + qw],
                        mask_sb[:, c0:c0 + qw],
                    )
                # PV: out^T (64 rows) + denominator (row 64)
                po = ps2.tile([HD + 1, 512], F32, tag="ps2")
                for t in range(nk):
                    g0, j = divmod(t, gsz)
                    b, c = j % 2, (j // 2) * qw
                    nc.tensor.matmul(
                        po[:, :qw],
                        lhsT=v_sb[:, t, (HD + 1) * h:(HD + 1) * (h + 1)],
                        rhs=p_tiles[g0][:, b, c:c + qw],
                        start=(t == 0), stop=(t == nk - 1),
                    )
                # normalize + stage this q block immediately. The denominator
                # row is reshaped across 128 partitions via a DRAM bounce so
                # the reciprocal runs partition-parallel ([128, qw/128]).
                un = upool.tile([HD + 1, 1024], F32, tag="unorm")
                nc.vector.tensor_copy(un[:, :qw], po[:, :qw])
                doff = (h * NQ + qb) * QB + q0
                cw = qw // 128
                nc.gpsimd.dma_start(
                    out=den_dram[h, qb, q0:q0 + qw], in_=un[HD:HD + 1, :qw]
                )
                rb = norm.tile([128, 8], F32, tag="rb")
                nc.gpsimd.dma_start(
                    out=rb[:, 0:cw],
                    in_=bass.AP(tensor=den_dram, offset=doff, ap=[[cw, 128], [1, cw]]),
                )
                nc.vector.reciprocal(rb[:, 4:4 + cw], rb[:, 0:cw])
                nc.gpsimd.dma_start(
                    out=bass.AP(tensor=rden_dram, offset=doff, ap=[[cw, 128], [1, cw]]),
                    in_=rb[:, 4:4 + cw],
                )
                bc = norm.tile([HD, 512], F32, tag="bcast")
                src = bass.AP(
                    tensor=rden_dram,
                    offset=(h * NQ + qb) * QB + q0,
                    ap=[[0, HD], [1, qw]],
                )
                nc.sync.dma_start(out=bc[:, :qw], in_=src)
                st = norm.tile([HD, 512], BF16, tag="stage")
                nc.vector.tensor_mul(st[:, :qw], un[0:HD, :qw], bc[:, :qw])
                if h == 0:
                    nc.sync.dma_start(out=a2a_in0[qb], in_=st[:, :qw])
                else:
                    wi = next(i for i, (w0, ww) in enumerate(WIN1) if w0 == q0)
                    nc.sync.dma_start(out=a2a_in1[wi][qb], in_=st[:, :qw])

            # ---- head-0 phase: proj block qb, then attention (h0, qb) ----
            load_x(0)
            load_x(1)
            for qb in range(NQ):
                proj_block(qb)
                load_x(qb + 2)
                attn_block(0, qb, 0, QB, 2)

            nc.gpsimd.collective_compute(
                "AllToAll",
                mybir.AluOpType.bypass,
                replica_groups=[list(range(N_CORES))],
                ins=[a2a_in0[:]],
                outs=[a2a_out0[:]],
            )
            nc.sync.dma_start(
                out=ao[0:HD, :, :], in_=a2a_out0.rearrange("g p s -> p g s"),
            )

            # ---- head-1 phase: windows, each its own A2A ----
            for w, (q0, qw) in enumerate(WIN1):
                for qb in range(NQ):
                    attn_block(1, qb, q0, qw, 1024 // qw)
                nc.gpsimd.collective_compute(
                    "AllToAll",
                    mybir.AluOpType.bypass,
                    replica_groups=[list(range(N_CORES))],
                    ins=[a2a_in1[w][:]],
                    outs=[a2a_out1[w][:]],
                )
                nc.sync.dma_start(
                    out=ao[HD:128, :, q0:q0 + qw],
                    in_=a2a_out1[w].rearrange("g p s -> p g s"),
                )

            # ---- output projection on local QB rows, split at K=64 ----
            # head-0 terms depend only on a2a_out0 (early); head-1 terms on
            # the window covering this m-subtile's columns.
            for m in range(MT):
                mo = 128 * m
                ob = work.tile([128, D], F32, tag="osb")
                for nh in range(2):
                    pf = ps1.tile([128, 1024], F32, tag="ps1")
                    for g in range(NQ):
                        nc.tensor.matmul(
                            pf[:, 0:512],
                            lhsT=ao[:, g, mo:mo + 128],
                            rhs=wp_sb[:, g, 512 * nh:512 * (nh + 1)],
                            start=(g == 0), stop=(g == NQ - 1),
                        )
                    nc.vector.tensor_add(
                        ob[:, 512 * nh:512 * (nh + 1)], pf[:, 0:512],
                        bp_full[:, 512 * nh:512 * (nh + 1)],
                    )
                nc.sync.dma_start(out=out_ext[128 * m:128 * (m + 1), :], in_=ob[:])

    nc.compile()
    return nc


def make_in_maps(S, x, w_qkv, b_qkv, w_proj, b_proj):
    """Host-side sharding: returns per-core input dicts (bf16-cast)."""
    x2 = np.ascontiguousarray(x.reshape(S, D))
    xT = np.ascontiguousarray(x2.T).astype(bf16)
    wproj_b = w_proj.astype(bf16)
    bproj_b = b_proj.reshape(1, D).astype(bf16)
    i, j = np.indices((128, 1024))
    mask = (i <= j - 384).astype(bf16)
    in_maps = []
    for c in range(N_CORES):
        cols = []
        bcols = []
        for part in range(3):  # q, k, v
            for hh in range(HPC):
                h = HPC * c + hh
                lo = part * D + HD * h
                cols.append(w_qkv[:, lo:lo + HD])
                bcols.append(b_qkv[lo:lo + HD])
        w_c = np.concatenate(cols, axis=1).astype(bf16)
        b_c = np.concatenate(bcols).reshape(1, MQKV).astype(bf16)
        in_maps.append({
            "xT": xT,
            "wqkv": np.ascontiguousarray(w_c),
            "bqkv": np.ascontiguousarray(b_c),
            "bqcol": np.ascontiguousarray(
                b_c.astype(np.float32).reshape(3, 128).T
            ),
            "wproj": wproj_b,
            "bproj": bproj_b,
            "mask": np.ascontiguousarray(mask),
            "salt": np.zeros((1, BUILD_SALT), np.float32),
        })
    return in_maps


_CACHE = {}


def _get_nc(S):
    if S not in _CACHE:
        _CACHE[S] = build(S)
    return _CACHE[S]


def kernel(x, w_qkv, b_qkv, w_proj, b_proj, trace=False):
    x = np.asarray(x, dtype=np.float32)
    w_qkv = np.asarray(w_qkv, dtype=np.float32)
    b_qkv = np.asarray(b_qkv, dtype=np.float32)
    w_proj = np.asarray(w_proj, dtype=np.float32)
    b_proj = np.asarray(b_proj, dtype=np.float32)
    B, S, _ = x.shape
    nc = _get_nc(S)
    in_maps = make_in_maps(S, x, w_qkv, b_qkv, w_proj, b_proj)
    res = run_bass_kernel_spmd(nc, in_maps, core_ids=list(range(N_CORES)), trace=trace)
    QB = S // N_CORES
    out = np.empty((S, D), dtype=np.float32)
    for c in range(N_CORES):
        out[QB * c:QB * (c + 1)] = res.results[c]["out"]
    if trace:
        kernel.last_exec_time_ns = res.exec_time_ns
        kernel.last_result = res
    return out.reshape(B, S, D)


# revision 46
# speedup vs baseline: 1.1967x; 1.1967x over previous
"""Distributed causal attention for TRN2 (8 NeuronCores).

Reference op (per core-external semantics):
    qkv = x @ w_qkv + b_qkv ; split into per-head q,k,v (16 heads, hd=64)
    causal softmax(q k^T / 8) v per head ; concat heads ; out = . @ w_proj + b_proj

Sharding: head-parallel attention (2 heads/core), AllToAll redistribution to
sequence-parallel for the output projection (each core owns S/8 query rows).

v2 structure (vs v1):
  - qkv projection is emitted per-seq-block interleaved with head-0 attention
    so the Scalar engine (exp, the critical resource) starts early.
  - QK^T matmuls alternate PE row-groups (even tiles use the head's natural
    64 partitions, odd tiles a DMA-duplicated copy in the opposite half) so
    consecutive K=64 matmuls run concurrently in the PE array.
  - softmax normalize+stage happens per q-block (reciprocal_approx_fast),
    not per window, so the A2As fire immediately after their last block.
  - head 1 runs in two 256-column windows with separate A2As; the output
    projection is split at K=64 so head-0 terms accumulate during the last
    A2A and only head-1-w1 terms remain after it.
  - qkv bias is applied by the DVE during PSUM->SBUF eviction (per-partition
    scalar), not by K=1 matmuls.

All matmuls run in bf16 (fp32 PSUM accumulation); softmax runs without
max-subtraction (scores are bounded for this problem's scale), with
denominators via a ones-column appended to V.

kernel(**inputs) takes the FULL fp32 inputs and returns the FULL fp32 output.
"""

import os

import numpy as np
import ml_dtypes

# comma-separated debug kill-switches, e.g. KBISECT=recip,bias,dup
KBISECT = set(filter(None, os.environ.get("KBISECT", "").split(",")))

import concourse.bacc as bacc
import concourse.bass as bass
import concourse.tile as tile
from concourse import masks, mybir
from concourse.bass_utils import run_bass_kernel_spmd

N_CORES = 8
D = 1024
H = 16
HD = 64
HPC = H // N_CORES          # heads per core = 2
MQKV = 3 * HPC * HD         # per-core qkv feature cols = 384

BF16 = mybir.dt.bfloat16
F32 = mybir.dt.float32
bf16 = ml_dtypes.bfloat16

# Bumping this changes the compiled executable's signature (a dummy input's
# shape encodes it), forcing a fresh compile + stage. Bump if a crashed run
# leaves a poisoned staged executable behind.
BUILD_SALT = 16 + sum(len(f) for f in KBISECT)


def build(S):
    QB = S // N_CORES        # query rows per core (A2A shard) = 512
    NQ = N_CORES             # number of q blocks == cores
    SKT = S // 128           # total sk tiles
    NPROJ = S // 512         # qkv-proj seq blocks of 512
    MT = QB // 128           # out-row subtiles of 128

    nc = bacc.Bacc("TRN2", num_devices=N_CORES)

    xT = nc.declare_dram_parameter("xT", [D, S], BF16, isOutput=False)
    wqkv = nc.declare_dram_parameter("wqkv", [D, MQKV], BF16, isOutput=False)
    bqkv = nc.declare_dram_parameter("bqkv", [1, MQKV], BF16, isOutput=False)
    bqcol = nc.declare_dram_parameter("bqcol", [128, 3], F32, isOutput=False)
    wproj = nc.declare_dram_parameter("wproj", [D, D], BF16, isOutput=False)
    bproj = nc.declare_dram_parameter("bproj", [1, D], BF16, isOutput=False)
    maskp = nc.declare_dram_parameter("mask", [128, 1024], BF16, isOutput=False)
    salt = nc.declare_dram_parameter("salt", [1, BUILD_SALT], F32, isOutput=False)
    out_ext = nc.declare_dram_parameter("out", [QB, D], F32, isOutput=True)

    # collective staging: head0 full-width; head1 in two 256-col windows
    a2a_in0 = nc.dram_tensor("a2a_in0", [NQ, HD, QB], BF16)
    a2a_out0 = nc.dram_tensor("a2a_out0", [NQ, HD, QB], BF16)
    W1 = QB if "v1tail" in KBISECT else QB // 2
    NW1 = QB // W1
    a2a_in1 = [nc.dram_tensor(f"a2a_in1_{w}", [NQ, HD, W1], BF16)
               for w in range(NW1)]
    a2a_out1 = [nc.dram_tensor(f"a2a_out1_{w}", [NQ, HD, W1], BF16)
                for w in range(NW1)]
    rden_dram = nc.dram_tensor("rden_dram", [HPC, NQ, QB], F32)
    den_dram = nc.dram_tensor("den_dram", [HPC, NQ, QB], F32)

    with tile.TileContext(nc) as tc:
        with (
            tc.tile_pool(name="singles", bufs=1) as singles,
            tc.tile_pool(name="xpool", bufs=3) as xpool,
            tc.tile_pool(name="work", bufs=2) as work,
            tc.tile_pool(name="norm", bufs=4) as norm,
            tc.tile_pool(name="ppool", bufs=8) as ppool,
            tc.tile_pool(name="upool", bufs=4) as upool,
            tc.tile_pool(name="ps1", bufs=3, space="PSUM") as ps1,
            tc.tile_pool(name="ps2", bufs=2, space="PSUM") as ps2,
        ):
            # ---- constants / weights ----
            w_sb = singles.tile([128, 8, MQKV], BF16)
            nc.sync.dma_start(out=w_sb[:], in_=wqkv.rearrange("(a p) m -> p a m", p=128))
            bq_sb = singles.tile([1, MQKV], BF16)
            nc.sync.dma_start(out=bq_sb[:], in_=bqkv[:])
            # bias as a per-partition f32 column for tensor_scalar eviction:
            # bq_col[p, m] = b_qkv[m*128 + p]
            bq_col = singles.tile([128, 3], F32)
            nc.sync.dma_start(out=bq_col[:], in_=bqcol[:])
            mask_sb = singles.tile([128, 1024], BF16)
            nc.sync.dma_start(out=mask_sb[:], in_=maskp[:])
            ones_sb = singles.tile([1, 512], BF16)
            nc.vector.memset(ones_sb[:], 1.0)
            ident = singles.tile([128, 128], BF16)
            masks.make_identity(nc, ident[:])
            wp_sb = singles.tile([128, 8, D], BF16)
            nc.sync.dma_start(out=wp_sb[:], in_=wproj.rearrange("(a p) m -> p a m", p=128))

            bp_sb = singles.tile([1, D], BF16)
            nc.sync.dma_start(out=bp_sb[:], in_=bproj[:])
            # b_proj broadcast to all 128 partitions (free-dim bias for the
            # output rows; folded into the PSUM eviction add)
            bp_full = singles.tile([128, D], BF16)
            nc.sync.dma_start(
                out=bp_full[:],
                in_=bass.AP(tensor=bproj, offset=0, ap=[[0, 128], [1, D]]),
            )
            salt_sb = singles.tile([1, BUILD_SALT], F32)
            nc.sync.dma_start(out=salt_sb[:], in_=salt[:])

            # persistent activation tensors
            qkvT = singles.tile([128, 3, S], BF16)   # [feat(2 heads), {q,k,v}, seq]
            # cross-duplicates: kdup rows 64:128 = k_h0, rows 0:64 = k_h1
            kdup = singles.tile([128, S], BF16)
            qdup = singles.tile([128, S], BF16)
            v_sb = singles.tile([128, SKT, 2 * (HD + 1)], BF16)
            nc.vector.memset(v_sb[:, :, HD:HD + 1], 1.0)
            nc.vector.memset(v_sb[:, :, 2 * HD + 1:2 * HD + 2], 1.0)
            # gathered A2A results: head 0 rows 0-63, head 1 rows 64-127
            ao = singles.tile([128, NQ, QB], BF16)

            xT_r = xT.rearrange("(a p) s -> p a s", p=128)
            x_tiles = {}

            def load_x(n):
                if n >= NPROJ:
                    return
                xs = xpool.tile([128, 8, 512], BF16, tag="x")
                for a in range(8):
                    nc.sync.dma_start(
                        out=xs[:, a, :], in_=xT_r[:, a, 512 * n:512 * (n + 1)]
                    )
                x_tiles[n] = xs

            def proj_block(n):
                """qkv^T projection for seq block n: qkvT[:, :, 512n:512n+512]."""
                xs = x_tiles.pop(n)
                for m in range(3):
                    ps = ps1.tile([128, 1024], F32, tag="ps1")
                    for a in range(8):
                        nc.tensor.matmul(
                            ps[:, 0:512],
                            lhsT=w_sb[:, a, 128 * m:128 * (m + 1)],
                            rhs=xs[:, a, :],
                            start=(a == 0), stop=(a == 7),
                        )
                    nc.vector.tensor_scalar_add(
                        qkvT[:, m, 512 * n:512 * (n + 1)], ps[:, 0:512],
                        bq_col[:, m:m + 1],
                    )
                # cross-duplicate k and q halves for PE row-group alternation
                if "dup" not in KBISECT:
                    nc.gpsimd.dma_start(
                        out=kdup[64:128, 512 * n:512 * (n + 1)],
                        in_=qkvT[0:64, 1, 512 * n:512 * (n + 1)],
                    )
                    nc.gpsimd.dma_start(
                        out=kdup[0:64, 512 * n:512 * (n + 1)],
                        in_=qkvT[64:128, 1, 512 * n:512 * (n + 1)],
                    )
                    nc.gpsimd.dma_start(
                        out=qdup[64:128, 512 * n:512 * (n + 1)],
                        in_=qkvT[0:64, 0, 512 * n:512 * (n + 1)],
                    )
                    nc.gpsimd.dma_start(
                        out=qdup[0:64, 512 * n:512 * (n + 1)],
                        in_=qkvT[64:128, 0, 512 * n:512 * (n + 1)],
                    )
                # V natural layout for the 4 new sk tiles
                for t in range(4 * n, 4 * n + 4):
                    pt = ps2.tile([128, 128], BF16, tag="ps2")
                    nc.tensor.transpose(pt[:], qkvT[:, 2, 128 * t:128 * (t + 1)], ident[:])
                    nc.vector.tensor_copy(v_sb[:, t, 0:HD], pt[:, 0:HD])
                    nc.vector.tensor_copy(v_sb[:, t, HD + 1:2 * HD + 1], pt[:, HD:2 * HD])

            def k_ap(h, t):
                """lhsT for QK^T: head h, sk tile t, alternating row halves."""
                lo, hi = (0, 64) if h == 0 else (64, 128)
                if t % 2 == 0 or "dup" in KBISECT:
                    return qkvT[lo:hi, 1, 128 * t:128 * (t + 1)]
                olo = 64 - lo
                return kdup[olo:olo + 64, 128 * t:128 * (t + 1)]

            def q_ap(h, t, c0, cw):
                """rhs for QK^T: head h q cols [c0, c0+cw), matching k_ap rows."""
                lo, hi = (0, 64) if h == 0 else (64, 128)
                if t % 2 == 0 or "dup" in KBISECT:
                    return qkvT[lo:hi, 0, c0:c0 + cw]
                olo = 64 - lo
                return qdup[olo:olo + 64, c0:c0 + cw]

            def attn_block(h, qb, q0, qw, gsz):
                """Attention for head h, q cols [qb*QB+q0, +qw); gsz sk-tiles
                per exp group (gsz*qw == 1024). Returns staged output."""
                nk = (qb * QB + q0 + qw) // 128  # causal sk tiles
                dstart = (qb * QB + q0) // 128   # first (partially) masked tile
                # tile j of a group goes to PSUM bank j%2 at column (j//2)*qw:
                # consecutive tiles alternate PE row-groups and run
                # concurrently, so they must drain into different banks.
                p_tiles = []
                for g0 in range(0, nk, gsz):
                    gw = min(gsz, nk - g0)   # gw is always 2 or 4
                    ps = ps1.tile([128, 2, 512], F32, tag="ps1")
                    for j in range(gw):
                        t = g0 + j
                        b, c = j % 2, (j // 2) * qw
                        nc.tensor.matmul(
                            ps[:, b, c:c + qw],
                            lhsT=k_ap(h, t),
                            rhs=q_ap(h, t, qb * QB + q0, qw),
                            start=True, stop=True,
                        )
                    pt = ppool.tile([128, 2, 512], BF16, tag="p")
                    cols = (gw + 1) // 2 * qw
                    nc.scalar.activation(
                        pt[:, :, 0:cols], ps[:, :, 0:cols],
                        mybir.ActivationFunctionType.Exp, scale=0.125,
                    )
                    p_tiles.append(pt)
                # causal mask on diagonal tiles: tile dstart+d needs
                # mask[r, j] = (r <= j - 128*d) over j in [0, qw)
                for t in range(dstart, nk):
                    d = t - dstart
                    g0, j = divmod(t, gsz)
                    b, c = j % 2, (j // 2) * qw
                    c0 = 384 - 128 * d
                    nc.vector.tensor_mul(
                        p_tiles[g0][:, b, c:c + qw], p_tiles[g0][:, b, c:c + qw],
                        mask_sb[:, c0:c0 + qw],
                    )
                # PV: out^T (64 rows) + denominator (row 64)
                po = ps2.tile([HD + 1, 512], F32, tag="ps2")
                for t in range(nk):
                    g0, j = divmod(t, gsz)
                    b, c = j % 2, (j // 2) * qw
                    nc.tensor.matmul(
                        po[:, :qw],
                        lhsT=v_sb[:, t, (HD + 1) * h:(HD + 1) * (h + 1)],
                        rhs=p_tiles[g0][:, b, c:c + qw],
                        start=(t == 0), stop=(t == nk - 1),
                    )
                # normalize + stage this q block immediately. The denominator
                # row is reshaped across 128 partitions via a DRAM bounce so
                # the reciprocal runs partition-parallel ([128, qw/128]).
                un = upool.tile([HD + 1, 1024], F32, tag="unorm")
                nc.vector.tensor_copy(un[:, :qw], po[:, :qw])
                doff = (h * NQ + qb) * QB + q0
                cw = qw // 128
                nc.gpsimd.dma_start(
                    out=den_dram[h, qb, q0:q0 + qw], in_=un[HD:HD + 1, :qw]
                )
                rb = norm.tile([128, 8], F32, tag="rb")
                nc.gpsimd.dma_start(
                    out=rb[:, 0:cw],
                    in_=bass.AP(tensor=den_dram, offset=doff, ap=[[cw, 128], [1, cw]]),
                )
                nc.vector.reciprocal(rb[:, 4:4 + cw], rb[:, 0:cw])
                nc.gpsimd.dma_start(
                    out=bass.AP(tensor=rden_dram, offset=doff, ap=[[cw, 128], [1, cw]]),
                    in_=rb[:, 4:4 + cw],
                )
                bc = norm.tile([HD, 512], F32, tag="bcast")
                src = bass.AP(
                    tensor=rden_dram,
                    offset=(h * NQ + qb) * QB + q0,
                    ap=[[0, HD], [1, qw]],
                )
                nc.sync.dma_start(out=bc[:, :qw], in_=src)
                st = norm.tile([HD, 512], BF16, tag="stage")
                nc.vector.tensor_mul(st[:, :qw], un[0:HD, :qw], bc[:, :qw])
                if h == 0:
                    nc.sync.dma_start(out=a2a_in0[qb], in_=st[:, :qw])
                else:
                    nc.sync.dma_start(out=a2a_in1[q0 // W1][qb], in_=st[:, :qw])

            # ---- head-0 phase: proj block qb, then attention (h0, qb) ----
            load_x(0)
            load_x(1)
            for qb in range(NQ):
                proj_block(qb)
                load_x(qb + 2)
                attn_block(0, qb, 0, QB, 2)

            nc.gpsimd.collective_compute(
                "AllToAll",
                mybir.AluOpType.bypass,
                replica_groups=[list(range(N_CORES))],
                ins=[a2a_in0[:]],
                outs=[a2a_out0[:]],
            )
            nc.sync.dma_start(
                out=ao[0:HD, :, :], in_=a2a_out0.rearrange("g p s -> p g s"),
            )

            # ---- head-1 phase: windows, each its own A2A ----
            for w in range(NW1):
                for qb in range(NQ):
                    attn_block(1, qb, W1 * w, W1, 1024 // W1)
                nc.gpsimd.collective_compute(
                    "AllToAll",
                    mybir.AluOpType.bypass,
                    replica_groups=[list(range(N_CORES))],
                    ins=[a2a_in1[w][:]],
                    outs=[a2a_out1[w][:]],
                )
                nc.sync.dma_start(
                    out=ao[HD:128, :, W1 * w:W1 * (w + 1)],
                    in_=a2a_out1[w].rearrange("g p s -> p g s"),
                )

            # ---- output projection on local QB rows, split at K=64 ----
            # head-0 terms depend only on a2a_out0 (early); head-1 terms on
            # the window covering this m-subtile's columns.
            for m in range(MT):
                mo = 128 * m
                ob = work.tile([128, D], F32, tag="osb")
                for nh in range(2):
                    pf = ps1.tile([128, 1024], F32, tag="ps1")
                    for g in range(NQ):
                        nc.tensor.matmul(
                            pf[:, 0:512],
                            lhsT=ao[:, g, mo:mo + 128],
                            rhs=wp_sb[:, g, 512 * nh:512 * (nh + 1)],
                            start=(g == 0), stop=(g == NQ - 1),
                        )
                    nc.vector.tensor_add(
                        ob[:, 512 * nh:512 * (nh + 1)], pf[:, 0:512],
                        bp_full[:, 512 * nh:512 * (nh + 1)],
                    )
                nc.sync.dma_start(out=out_ext[128 * m:128 * (m + 1), :], in_=ob[:])

    nc.compile()
    return nc


def make_in_maps(S, x, w_qkv, b_qkv, w_proj, b_proj):
    """Host-side sharding: returns per-core input dicts (bf16-cast)."""
    x2 = np.ascontiguousarray(x.reshape(S, D))
    xT = np.ascontiguousarray(x2.T).astype(bf16)
    wproj_b = w_proj.astype(bf16)
    bproj_b = b_proj.reshape(1, D).astype(bf16)
    i, j = np.indices((128, 1024))
    mask = (i <= j - 384).astype(bf16)
    in_maps = []
    for c in range(N_CORES):
        cols = []
        bcols = []
        for part in range(3):  # q, k, v
            for hh in range(HPC):
                h = HPC * c + hh
                lo = part * D + HD * h
                cols.append(w_qkv[:, lo:lo + HD])
                bcols.append(b_qkv[lo:lo + HD])
        w_c = np.concatenate(cols, axis=1).astype(bf16)
        b_c = np.concatenate(bcols).reshape(1, MQKV).astype(bf16)
        in_maps.append({
            "xT": xT,
            "wqkv": np.ascontiguousarray(w_c),
            "bqkv": np.ascontiguousarray(b_c),
            "bqcol": np.ascontiguousarray(
                b_c.astype(np.float32).reshape(3, 128).T
            ),
            "wproj": wproj_b,
            "bproj": bproj_b,
            "mask": np.ascontiguousarray(mask),
            "salt": np.zeros((1, BUILD_SALT), np.float32),
        })
    return in_maps


_CACHE = {}


def _get_nc(S):
    if S not in _CACHE:
        _CACHE[S] = build(S)
    return _CACHE[S]


def kernel(x, w_qkv, b_qkv, w_proj, b_proj, trace=False):
    x = np.asarray(x, dtype=np.float32)
    w_qkv = np.asarray(w_qkv, dtype=np.float32)
    b_qkv = np.asarray(b_qkv, dtype=np.float32)
    w_proj = np.asarray(w_proj, dtype=np.float32)
    b_proj = np.asarray(b_proj, dtype=np.float32)
    B, S, _ = x.shape
    nc = _get_nc(S)
    in_maps = make_in_maps(S, x, w_qkv, b_qkv, w_proj, b_proj)
    res = run_bass_kernel_spmd(nc, in_maps, core_ids=list(range(N_CORES)), trace=trace)
    QB = S // N_CORES
    out = np.empty((S, D), dtype=np.float32)
    for c in range(N_CORES):
        out[QB * c:QB * (c + 1)] = res.results[c]["out"]
    if trace:
        kernel.last_exec_time_ns = res.exec_time_ns
        kernel.last_result = res
    return out.reshape(B, S, D)


# revision 47
# speedup vs baseline: 1.2846x; 1.0734x over previous
"""Distributed causal attention for TRN2 (8 NeuronCores).

Reference op (per core-external semantics):
    qkv = x @ w_qkv + b_qkv ; split into per-head q,k,v (16 heads, hd=64)
    causal softmax(q k^T / 8) v per head ; concat heads ; out = . @ w_proj + b_proj

Sharding: head-parallel attention (2 heads/core), AllToAll redistribution to
sequence-parallel for the output projection (each core owns S/8 query rows).

v2 structure (vs v1):
  - qkv projection is emitted per-seq-block interleaved with head-0 attention
    so the Scalar engine (exp, the critical resource) starts early.
  - QK^T matmuls alternate PE row-groups (even tiles use the head's natural
    64 partitions, odd tiles a DMA-duplicated copy in the opposite half) so
    consecutive K=64 matmuls run concurrently in the PE array.
  - softmax normalize+stage happens per q-block (reciprocal_approx_fast),
    not per window, so the A2As fire immediately after their last block.
  - head 1 runs in two 256-column windows with separate A2As; the output
    projection is split at K=64 so head-0 terms accumulate during the last
    A2A and only head-1-w1 terms remain after it.
  - qkv bias is applied by the DVE during PSUM->SBUF eviction (per-partition
    scalar), not by K=1 matmuls.

All matmuls run in bf16 (fp32 PSUM accumulation); softmax runs without
max-subtraction (scores are bounded for this problem's scale), with
denominators via a ones-column appended to V.

kernel(**inputs) takes the FULL fp32 inputs and returns the FULL fp32 output.
"""

import os

import numpy as np
import ml_dtypes

# comma-separated debug kill-switches, e.g. KBISECT=recip,bias,dup
KBISECT = set(filter(None, os.environ.get("KBISECT", "").split(",")))

import concourse.bacc as bacc
import concourse.bass as bass
import concourse.tile as tile
from concourse import masks, mybir
from concourse.bass_utils import run_bass_kernel_spmd

N_CORES = 8
D = 1024
H = 16
HD = 64
HPC = H // N_CORES          # heads per core = 2
MQKV = 3 * HPC * HD         # per-core qkv feature cols = 384

BF16 = mybir.dt.bfloat16
F32 = mybir.dt.float32
bf16 = ml_dtypes.bfloat16

# Bumping this changes the compiled executable's signature (a dummy input's
# shape encodes it), forcing a fresh compile + stage. Bump if a crashed run
# leaves a poisoned staged executable behind.
BUILD_SALT = 19 + sum(len(f) for f in KBISECT)


def build(S):
    QB = S // N_CORES        # query rows per core (A2A shard) = 512
    NQ = N_CORES             # number of q blocks == cores
    SKT = S // 128           # total sk tiles
    NPROJ = S // 512         # qkv-proj seq blocks of 512
    MT = QB // 128           # out-row subtiles of 128

    nc = bacc.Bacc("TRN2", num_devices=N_CORES)

    xT = nc.declare_dram_parameter("xT", [D, S], BF16, isOutput=False)
    wqkv = nc.declare_dram_parameter("wqkv", [D, MQKV], BF16, isOutput=False)
    bqkv = nc.declare_dram_parameter("bqkv", [1, MQKV], BF16, isOutput=False)
    bqcol = nc.declare_dram_parameter("bqcol", [128, 3], F32, isOutput=False)
    wproj = nc.declare_dram_parameter("wproj", [D, D], BF16, isOutput=False)
    bproj = nc.declare_dram_parameter("bproj", [1, D], BF16, isOutput=False)
    maskp = nc.declare_dram_parameter("mask", [128, 1024], BF16, isOutput=False)
    salt = nc.declare_dram_parameter("salt", [1, BUILD_SALT], F32, isOutput=False)
    out_ext = nc.declare_dram_parameter("out", [QB, D], F32, isOutput=True)

    # collective staging: head0 full-width; head1 in two 256-col windows
    a2a_in0 = nc.dram_tensor("a2a_in0", [NQ, HD, QB], BF16)
    a2a_out0 = nc.dram_tensor("a2a_out0", [NQ, HD, QB], BF16)
    W1 = QB if "v1tail" in KBISECT else QB // 2
    NW1 = QB // W1
    a2a_in1 = [nc.dram_tensor(f"a2a_in1_{w}", [NQ, HD, W1], BF16)
               for w in range(NW1)]
    a2a_out1 = [nc.dram_tensor(f"a2a_out1_{w}", [NQ, HD, W1], BF16)
                for w in range(NW1)]
    rden_dram = nc.dram_tensor("rden_dram", [HPC, NQ, QB], F32)
    den_dram = nc.dram_tensor("den_dram", [HPC, NQ, QB], F32)

    with tile.TileContext(nc) as tc:
        with (
            tc.tile_pool(name="singles", bufs=1) as singles,
            tc.tile_pool(name="xpool", bufs=3) as xpool,
            tc.tile_pool(name="work", bufs=2) as work,
            tc.tile_pool(name="norm", bufs=4) as norm,
            tc.tile_pool(name="ppool", bufs=8) as ppool,
            tc.tile_pool(name="upool", bufs=4) as upool,
            tc.tile_pool(name="ps1", bufs=3, space="PSUM") as ps1,
            tc.tile_pool(name="ps2", bufs=2, space="PSUM") as ps2,
        ):
            # ---- constants / weights ----
            w_sb = singles.tile([128, 8, MQKV], BF16)
            nc.sync.dma_start(out=w_sb[:], in_=wqkv.rearrange("(a p) m -> p a m", p=128))
            bq_sb = singles.tile([1, MQKV], BF16)
            nc.sync.dma_start(out=bq_sb[:], in_=bqkv[:])
            # bias as a per-partition f32 column for tensor_scalar eviction:
            # bq_col[p, m] = b_qkv[m*128 + p]
            bq_col = singles.tile([128, 3], F32)
            nc.sync.dma_start(out=bq_col[:], in_=bqcol[:])
            mask_sb = singles.tile([128, 1024], BF16)
            nc.sync.dma_start(out=mask_sb[:], in_=maskp[:])
            ones_sb = singles.tile([1, 512], BF16)
            nc.vector.memset(ones_sb[:], 1.0)
            ident = singles.tile([128, 128], BF16)
            masks.make_identity(nc, ident[:])
            wp_sb = singles.tile([128, 8, D], BF16)
            nc.sync.dma_start(out=wp_sb[:], in_=wproj.rearrange("(a p) m -> p a m", p=128))

            bp_sb = singles.tile([1, D], BF16)
            nc.sync.dma_start(out=bp_sb[:], in_=bproj[:])
            # b_proj broadcast to all 128 partitions (free-dim bias for the
            # output rows; folded into the PSUM eviction add)
            bp_full = singles.tile([128, D], BF16)
            nc.sync.dma_start(
                out=bp_full[:],
                in_=bass.AP(tensor=bproj, offset=0, ap=[[0, 128], [1, D]]),
            )
            salt_sb = singles.tile([1, BUILD_SALT], F32)
            nc.sync.dma_start(out=salt_sb[:], in_=salt[:])

            # persistent activation tensors
            qkvT = singles.tile([128, 3, S], BF16)   # [feat(2 heads), {q,k,v}, seq]
            # cross-duplicates: kdup rows 64:128 = k_h0, rows 0:64 = k_h1
            kdup = singles.tile([128, S], BF16)
            qdup = singles.tile([128, S], BF16)
            v_sb = singles.tile([128, SKT, 2 * (HD + 1)], BF16)
            nc.vector.memset(v_sb[:, :, HD:HD + 1], 1.0)
            nc.vector.memset(v_sb[:, :, 2 * HD + 1:2 * HD + 2], 1.0)
            # gathered A2A results: head 0 rows 0-63, head 1 rows 64-127
            ao = singles.tile([128, NQ, QB], BF16)

            xT_r = xT.rearrange("(a p) s -> p a s", p=128)
            x_tiles = {}

            def load_x(n):
                if n >= NPROJ:
                    return
                xs = xpool.tile([128, 8, 512], BF16, tag="x")
                for a in range(8):
                    nc.sync.dma_start(
                        out=xs[:, a, :], in_=xT_r[:, a, 512 * n:512 * (n + 1)]
                    )
                x_tiles[n] = xs

            def proj_block(n):
                """qkv^T projection for seq block n: qkvT[:, :, 512n:512n+512]."""
                xs = x_tiles.pop(n)
                for m in range(3):
                    ps = ps1.tile([128, 1024], F32, tag="ps1")
                    for a in range(8):
                        nc.tensor.matmul(
                            ps[:, 0:512],
                            lhsT=w_sb[:, a, 128 * m:128 * (m + 1)],
                            rhs=xs[:, a, :],
                            start=(a == 0), stop=(a == 7),
                        )
                    nc.vector.tensor_scalar_add(
                        qkvT[:, m, 512 * n:512 * (n + 1)], ps[:, 0:512],
                        bq_col[:, m:m + 1],
                    )
                # cross-duplicate k and q halves for PE row-group alternation
                if "dup" not in KBISECT:
                    nc.gpsimd.dma_start(
                        out=kdup[64:128, 512 * n:512 * (n + 1)],
                        in_=qkvT[0:64, 1, 512 * n:512 * (n + 1)],
                    )
                    nc.gpsimd.dma_start(
                        out=kdup[0:64, 512 * n:512 * (n + 1)],
                        in_=qkvT[64:128, 1, 512 * n:512 * (n + 1)],
                    )
                    nc.gpsimd.dma_start(
                        out=qdup[64:128, 512 * n:512 * (n + 1)],
                        in_=qkvT[0:64, 0, 512 * n:512 * (n + 1)],
                    )
                    nc.gpsimd.dma_start(
                        out=qdup[0:64, 512 * n:512 * (n + 1)],
                        in_=qkvT[64:128, 0, 512 * n:512 * (n + 1)],
                    )
                # V natural layout for the 4 new sk tiles
                for t in range(4 * n, 4 * n + 4):
                    pt = ps2.tile([128, 128], BF16, tag="ps2")
                    nc.tensor.transpose(pt[:], qkvT[:, 2, 128 * t:128 * (t + 1)], ident[:])
                    nc.vector.tensor_copy(v_sb[:, t, 0:HD], pt[:, 0:HD])
                    nc.vector.tensor_copy(v_sb[:, t, HD + 1:2 * HD + 1], pt[:, HD:2 * HD])

            def k_ap(h, t):
                """lhsT for QK^T: head h, sk tile t, alternating row halves."""
                lo, hi = (0, 64) if h == 0 else (64, 128)
                if t % 2 == 0 or "dup" in KBISECT:
                    return qkvT[lo:hi, 1, 128 * t:128 * (t + 1)]
                olo = 64 - lo
                return kdup[olo:olo + 64, 128 * t:128 * (t + 1)]

            def q_ap(h, t, c0, cw):
                """rhs for QK^T: head h q cols [c0, c0+cw), matching k_ap rows."""
                lo, hi = (0, 64) if h == 0 else (64, 128)
                if t % 2 == 0 or "dup" in KBISECT:
                    return qkvT[lo:hi, 0, c0:c0 + cw]
                olo = 64 - lo
                return qdup[olo:olo + 64, c0:c0 + cw]

            def attn_block(h, qb, q0, qw, gsz):
                """Attention for head h, q cols [qb*QB+q0, +qw); gsz sk-tiles
                per exp group (gsz*qw == 1024). Returns staged output."""
                nk = (qb * QB + q0 + qw) // 128  # causal sk tiles
                dstart = (qb * QB + q0) // 128   # first (partially) masked tile
                # tile j of a group goes to PSUM bank j%2 at column (j//2)*qw:
                # consecutive tiles alternate PE row-groups and run
                # concurrently, so they must drain into different banks.
                p_tiles = []
                for g0 in range(0, nk, gsz):
                    gw = min(gsz, nk - g0)   # gw is always 2 or 4
                    ps = ps1.tile([128, 2, 512], F32, tag="ps1")
                    for j in range(gw):
                        t = g0 + j
                        b, c = j % 2, (j // 2) * qw
                        nc.tensor.matmul(
                            ps[:, b, c:c + qw],
                            lhsT=k_ap(h, t),
                            rhs=q_ap(h, t, qb * QB + q0, qw),
                            start=True, stop=True,
                        )
                    pt = ppool.tile([128, 2, 512], BF16, tag="p")
                    cols = (gw + 1) // 2 * qw
                    nc.scalar.activation(
                        pt[:, :, 0:cols], ps[:, :, 0:cols],
                        mybir.ActivationFunctionType.Exp, scale=0.125,
                    )
                    p_tiles.append(pt)
                # causal mask on diagonal tiles: tile dstart+d needs
                # mask[r, j] = (r <= j - 128*d) over j in [0, qw)
                for t in range(dstart, nk):
                    d = t - dstart
                    g0, j = divmod(t, gsz)
                    b, c = j % 2, (j // 2) * qw
                    c0 = 384 - 128 * d
                    nc.vector.tensor_mul(
                        p_tiles[g0][:, b, c:c + qw], p_tiles[g0][:, b, c:c + qw],
                        mask_sb[:, c0:c0 + qw],
                    )
                # PV: out^T (64 rows) + denominator (row 64)
                po = ps2.tile([HD + 1, 512], F32, tag="ps2")
                for t in range(nk):
                    g0, j = divmod(t, gsz)
                    b, c = j % 2, (j // 2) * qw
                    nc.tensor.matmul(
                        po[:, :qw],
                        lhsT=v_sb[:, t, (HD + 1) * h:(HD + 1) * (h + 1)],
                        rhs=p_tiles[g0][:, b, c:c + qw],
                        start=(t == 0), stop=(t == nk - 1),
                    )
                # normalize + stage this q block immediately. The denominator
                # row is reshaped across 128 partitions via a DRAM bounce so
                # the reciprocal runs partition-parallel ([128, qw/128]).
                un = upool.tile([HD + 1, 1024], F32, tag="unorm")
                nc.vector.tensor_copy(un[:, :qw], po[:, :qw])
                doff = (h * NQ + qb) * QB + q0
                cw = qw // 128
                nc.gpsimd.dma_start(
                    out=den_dram[h, qb, q0:q0 + qw], in_=un[HD:HD + 1, :qw]
                )
                rb = norm.tile([128, 8], F32, tag="rb")
                nc.gpsimd.dma_start(
                    out=rb[:, 0:cw],
                    in_=bass.AP(tensor=den_dram, offset=doff, ap=[[cw, 128], [1, cw]]),
                )
                nc.vector.reciprocal(rb[:, 4:4 + cw], rb[:, 0:cw])
                nc.gpsimd.dma_start(
                    out=bass.AP(tensor=rden_dram, offset=doff, ap=[[cw, 128], [1, cw]]),
                    in_=rb[:, 4:4 + cw],
                )
                bc = norm.tile([HD, 512], F32, tag="bcast")
                src = bass.AP(
                    tensor=rden_dram,
                    offset=(h * NQ + qb) * QB + q0,
                    ap=[[0, HD], [1, qw]],
                )
                nc.sync.dma_start(out=bc[:, :qw], in_=src)
                st = norm.tile([HD, 512], BF16, tag="stage")
                nc.vector.tensor_mul(st[:, :qw], un[0:HD, :qw], bc[:, :qw])
                if h == 0:
                    nc.sync.dma_start(out=a2a_in0[qb], in_=st[:, :qw])
                else:
                    nc.sync.dma_start(out=a2a_in1[q0 // W1][qb], in_=st[:, :qw])

            # ---- head-0 phase: proj block qb, then attention (h0, qb) ----
            load_x(0)
            load_x(1)
            for qb in range(NQ):
                proj_block(qb)
                load_x(qb + 2)
                attn_block(0, qb, 0, QB, 2)

            nc.gpsimd.collective_compute(
                "AllToAll",
                mybir.AluOpType.bypass,
                replica_groups=[list(range(N_CORES))],
                ins=[a2a_in0[:]],
                outs=[a2a_out0[:]],
            )

            # ---- head-1 phase: windows, each its own A2A ----
            for w in range(NW1):
                for qb in range(NQ):
                    attn_block(1, qb, W1 * w, W1, 1024 // W1)
                nc.gpsimd.collective_compute(
                    "AllToAll",
                    mybir.AluOpType.bypass,
                    replica_groups=[list(range(N_CORES))],
                    ins=[a2a_in1[w][:]],
                    outs=[a2a_out1[w][:]],
                )

            # A2A result gathers, emitted late so their wait on the
            # collectives doesn't block earlier DMAs in the queue FIFO
            nc.sync.dma_start(
                out=ao[0:HD, :, :], in_=a2a_out0.rearrange("g p s -> p g s"),
            )
            for w in range(NW1):
                nc.sync.dma_start(
                    out=ao[HD:128, :, W1 * w:W1 * (w + 1)],
                    in_=a2a_out1[w].rearrange("g p s -> p g s"),
                )

            # ---- output projection on local QB rows, split at K=64 ----
            # head-0 terms depend only on a2a_out0 (early); head-1 terms on
            # the window covering this m-subtile's columns.
            for m in range(MT):
                mo = 128 * m
                ob = work.tile([128, D], F32, tag="osb")
                for nh in range(2):
                    pf = ps1.tile([128, 1024], F32, tag="ps1")
                    for g in range(NQ):
                        nc.tensor.matmul(
                            pf[:, 0:512],
                            lhsT=ao[:, g, mo:mo + 128],
                            rhs=wp_sb[:, g, 512 * nh:512 * (nh + 1)],
                            start=(g == 0), stop=(g == NQ - 1),
                        )
                    nc.vector.tensor_add(
                        ob[:, 512 * nh:512 * (nh + 1)], pf[:, 0:512],
                        bp_full[:, 512 * nh:512 * (nh + 1)],
                    )
                nc.sync.dma_start(out=out_ext[128 * m:128 * (m + 1), :], in_=ob[:])

    nc.compile()
    return nc


def make_in_maps(S, x, w_qkv, b_qkv, w_proj, b_proj):
    """Host-side sharding: returns per-core input dicts (bf16-cast)."""
    x2 = np.ascontiguousarray(x.reshape(S, D))
    xT = np.ascontiguousarray(x2.T).astype(bf16)
    wproj_b = w_proj.astype(bf16)
    bproj_b = b_proj.reshape(1, D).astype(bf16)
    i, j = np.indices((128, 1024))
    mask = (i <= j - 384).astype(bf16)
    in_maps = []
    for c in range(N_CORES):
        cols = []
        bcols = []
        for part in range(3):  # q, k, v
            for hh in range(HPC):
                h = HPC * c + hh
                lo = part * D + HD * h
                cols.append(w_qkv[:, lo:lo + HD])
                bcols.append(b_qkv[lo:lo + HD])
        w_c = np.concatenate(cols, axis=1).astype(bf16)
        b_c = np.concatenate(bcols).reshape(1, MQKV).astype(bf16)
        in_maps.append({
            "xT": xT,
            "wqkv": np.ascontiguousarray(w_c),
            "bqkv": np.ascontiguousarray(b_c),
            "bqcol": np.ascontiguousarray(
                b_c.astype(np.float32).reshape(3, 128).T
            ),
            "wproj": wproj_b,
            "bproj": bproj_b,
            "mask": np.ascontiguousarray(mask),
            "salt": np.zeros((1, BUILD_SALT), np.float32),
        })
    return in_maps


_CACHE = {}


def _get_nc(S):
    if S not in _CACHE:
        _CACHE[S] = build(S)
    return _CACHE[S]


def kernel(x, w_qkv, b_qkv, w_proj, b_proj, trace=False):
    x = np.asarray(x, dtype=np.float32)
    w_qkv = np.asarray(w_qkv, dtype=np.float32)
    b_qkv = np.asarray(b_qkv, dtype=np.float32)
    w_proj = np.asarray(w_proj, dtype=np.float32)
    b_proj = np.asarray(b_proj, dtype=np.float32)
    B, S, _ = x.shape
    nc = _get_nc(S)
    in_maps = make_in_maps(S, x, w_qkv, b_qkv, w_proj, b_proj)
    res = run_bass_kernel_spmd(nc, in_maps, core_ids=list(range(N_CORES)), trace=trace)
    QB = S // N_CORES
    out = np.empty((S, D), dtype=np.float32)
    for c in range(N_CORES):
        out[QB * c:QB * (c + 1)] = res.results[c]["out"]
    if trace:
        kernel.last_exec_time_ns = res.exec_time_ns
        kernel.last_result = res
    return out.reshape(B, S, D)
